# revision 21
# baseline (speedup 1.0000x reference)
"""ConvCNP Trainium2 kernel (8 NeuronCores, SPMD, no collectives).

Strategy: the context/query kernels are Laplacian, exp(-0.5|dx|/l) with
l = ql = 1e-3, so influence decays below 1.4e-11 at |dx| = 0.05. Targets
are sorted on the host and split into 8 equal shards; each core works on
a contiguous grid *window* (multiple of 128 rows) that covers its targets
plus the kernel band and the 6-row CNN halo. Context points are restricted
to the window's band and padded with far-away sentinels (whose kernel
weights underflow to exactly 0). Everything else runs densely on-chip:

  phase A: w = exp(-0.5|xc - t|/l) per 128-context tile (DVE fused
           sub+abs, ACT exp), PE-matmul against [y_c | valid] to get
           [signal; density] in PSUM [2, W].
  phase B: 3-layer CNN (k=5, SAME) as 5 shifted-slice matmuls per layer;
           layer 3 is emitted with grid on partitions so the grid encoding
           lands directly as [128, 2] tiles (no transposes anywhere).
  phase C: bases = exp(-0.5|xt - t|/ql) per 128-grid-row tile, PE-matmul
           against the grid encoding -> [2, 2048] (means; raw stds),
           exp on the stds row, DMA out.

Host gathers per-core [2, 2048] outputs, transposes, and undoes the sort.
"""

import os

import numpy as np

import concourse.bacc as bacc
import concourse.bass as bass
import concourse.mybir as mybir
from concourse.tile import TileContext

GRID_SPACING = 0.0005
EPS = 1e-8
NCORES = 8
P = 128
PAD = 0.05  # kernel band cutoff: exp(-0.5*PAD/l) ~ 1.4e-11
HALO = 6  # 3 conv layers x (k//2) rows of edge contamination
SENTINEL = 1.0e6
TCHUNK = 512  # matmul moving-free-dim / PSUM-bank limit

LAST_RESULT = None  # BassKernelResults of the most recent hardware run


def _round_up(x, m):
    return ((x + m - 1) // m) * m


def _build_program(W, NCT, NGT, TL, scale_c, scale_q, sim_abs=False):
    """Build the single SPMD Bass/Tile program (shapes/scales baked)."""
    f32 = mybir.dt.float32
    AF = mybir.ActivationFunctionType
    OP = mybir.AluOpType

    # Bacc (not raw Bass): its compile pipeline legalizes sync waits —
    # walrus's per-instruction structs carry at most ONE wait, and Bacc's
    # generate_event_semaphores/move_matmul_waits_to_ldweights passes split
    # multi-sem joins that Tile emits freely.
    nc = bacc.Bacc("TRN2", target_bir_lowering=False, debug=False)

    xc_cols_d = nc.dram_tensor("xc_cols", [P, NCT], f32, kind="ExternalInput")
    yc1_d = nc.dram_tensor("yc1", [P, 33 * NCT], f32, kind="ExternalInput")
    tb_d = nc.dram_tensor("t_bcast", [P, W], f32, kind="ExternalInput")
    tcol_d = nc.dram_tensor("t_cols", [P, NGT], f32, kind="ExternalInput")
    xt_d = nc.dram_tensor("xt_bcast", [P, TL], f32, kind="ExternalInput")
    w1_d = nc.dram_tensor("w1T", [2, 5 * 32], f32, kind="ExternalInput")
    w2_d = nc.dram_tensor("w2T", [32, 5 * 32], f32, kind="ExternalInput")
    w3_d = nc.dram_tensor("w3T", [32, 5 * 2], f32, kind="ExternalInput")
    b1_d = nc.dram_tensor("b1c", [32, 1], f32, kind="ExternalInput")
    b2_d = nc.dram_tensor("b2c", [32, 1], f32, kind="ExternalInput")
    b3_d = nc.dram_tensor("b3r", [P, 2], f32, kind="ExternalInput")
    out_d = nc.dram_tensor("out", [2, TL], f32, kind="ExternalOutput")

    # free-dim chunks of <=512 covering the window (matmul N limit)
    wchunks = [(o, min(TCHUNK, W - o)) for o in range(0, W, TCHUNK)]
    acc_bufs = 2 if W <= TCHUNK else 1  # keep PSUM within 8 banks
    TCH = TL // TCHUNK

    with TileContext(nc) as tcx:
        with (
            tcx.tile_pool(name="const", bufs=1) as cp,
            tcx.tile_pool(name="work", bufs=NCT) as wp,
            tcx.tile_pool(name="qwork", bufs=NGT) as qp,
            tcx.tile_pool(name="pp", bufs=1, space="PSUM") as pp,
        ):
            i32 = mybir.dt.int32
            xc_sb = cp.tile_from(xc_cols_d.ap())
            yc1_sb = cp.tile_from(yc1_d.ap())
            t_sb = cp.tile_from(tb_d.ap())
            tcol_sb = cp.tile_from(tcol_d.ap())
            xt_sb = cp.tile_from(xt_d.ap())
            # fp32 |x| = clear sign bit: AND with 0x7fffffff on int32 views
            # (abs_max/bitwise can't fuse into one TensorScalar on this ISA;
            # CoreSim conversely can't run int-scalar TS, so sim_abs swaps in
            # an abs_max-vs-0.0 TS of identical shape)
            mask_sb = cp.tile([P, 1], i32)
            nc.vector.memset(mask_sb[:], 0x7FFFFFFF)

            def _abs_ts(dst, src):
                if sim_abs:
                    nc.vector.tensor_scalar(
                        dst[:], src[:], 0.0, None, OP.abs_max
                    )
                else:
                    nc.vector.tensor_scalar(
                        dst[:].bitcast(i32), src[:].bitcast(i32),
                        mask_sb[:, 0:1], None, OP.bitwise_and,
                    )
            w1_sb = cp.tile_from(w1_d.ap())
            w2_sb = cp.tile_from(w2_d.ap())
            w3_sb = cp.tile_from(w3_d.ap())
            b1_sb = cp.tile_from(b1_d.ap())
            b2_sb = cp.tile_from(b2_d.ap())
            b3_sb = cp.tile_from(b3_d.ap())

            # ---- phase A: context encoding. Signal lands on partition 0,
            # density on partition 32 (compute engines can only address
            # partition starts in {0,32,64,96}, so M=33 with zero middle).
            ph = pp.tile([33, W], f32, name="ph", tag="acc", bufs=acc_bufs)
            for i in range(NCT):
                d = wp.tile([P, W], f32, name="d")
                nc.vector.tensor_scalar(
                    d[:], t_sb[:], xc_sb[:, i : i + 1], None, OP.subtract
                )
                da = wp.tile([P, W], f32, name="da")
                _abs_ts(da, d)
                wex = wp.tile([P, W], f32, name="wex")
                nc.scalar.activation(wex[:], da[:], AF.Exp, scale=scale_c)
                for (o, ln) in wchunks:
                    nc.tensor.matmul(
                        ph[:, o : o + ln],
                        yc1_sb[:, 33 * i : 33 * i + 33],
                        wex[:, o : o + ln],
                        start=(i == 0),
                        stop=(i == NCT - 1),
                    )

            # ---- h = [density; signal/(density+eps)], zero-padded by 2.
            # Rows are built at partition 0 and DMA'd into place (DMA has
            # no partition-start restriction).
            h1 = cp.tile([2, W + 4], f32)
            nc.vector.memset(h1[:], 0.0)
            eps_sb = cp.tile([1, 1], f32)
            nc.vector.memset(eps_sb[:], EPS)
            de = cp.tile([1, W], f32)
            nc.scalar.activation(
                de[:], ph[32:33, :], AF.Identity, bias=eps_sb[0:1, 0:1]
            )
            rc = cp.tile([1, W], f32)
            nc.vector.reciprocal(rc[:], de[:])
            dens = cp.tile([1, W], f32)
            nc.scalar.copy(dens[:], ph[32:33, :])
            ratio = cp.tile([1, W], f32)
            nc.vector.tensor_mul(ratio[:], ph[0:1, :], rc[:])
            nc.sync.dma_start(h1[0:1, 2 : W + 2], dens[:])
            nc.sync.dma_start(h1[1:2, 2 : W + 2], ratio[:])

            # ---- phase B: CNN. layers 1-2: channels on partitions
            p1 = pp.tile([32, W], f32, name="p1", tag="acc", bufs=acc_bufs)
            for k in range(5):
                for (o, ln) in wchunks:
                    nc.tensor.matmul(
                        p1[:, o : o + ln],
                        w1_sb[:, 32 * k : 32 * k + 32],
                        h1[:, k + o : k + o + ln],
                        start=(k == 0),
                        stop=(k == 4),
                    )
            h2 = cp.tile([32, W + 4], f32)
            nc.vector.memset(h2[:], 0.0)
            nc.scalar.activation(h2[:, 2 : W + 2], p1[:], AF.Relu, bias=b1_sb[:])

            p2 = pp.tile([32, W], f32, name="p2", tag="acc", bufs=acc_bufs)
            for k in range(5):
                for (o, ln) in wchunks:
                    nc.tensor.matmul(
                        p2[:, o : o + ln],
                        w2_sb[:, 32 * k : 32 * k + 32],
                        h2[:, k + o : k + o + ln],
                        start=(k == 0),
                        stop=(k == 4),
                    )
            h3 = cp.tile([32, W + 4], f32)
            nc.vector.memset(h3[:], 0.0)
            nc.scalar.activation(h3[:, 2 : W + 2], p2[:], AF.Relu, bias=b2_sb[:])

            # layer 3 flipped: grid rows on partitions. Grid-encoding
            # channel 0 goes to column 33j, channel 1 to column 33j+32 so
            # the query matmul puts means on psum partition 0 and raw stds
            # on partition 32.
            gencT = cp.tile([P, 33 * NGT], f32)
            nc.vector.memset(gencT[:], 0.0)
            for j in range(NGT):
                p3 = pp.tile([P, 2], f32, name="p3", tag="l3", bufs=2)
                for k in range(5):
                    nc.tensor.matmul(
                        p3[:],
                        h3[:, P * j + k : P * j + k + P],
                        w3_sb[:, 2 * k : 2 * k + 2],
                        start=(k == 0),
                        stop=(k == 4),
                    )
                nc.vector.tensor_add(
                    gencT[:, 33 * j : 33 * j + 1], p3[:, 0:1], b3_sb[:, 0:1]
                )
                nc.vector.tensor_add(
                    gencT[:, 33 * j + 32 : 33 * j + 33], p3[:, 1:2], b3_sb[:, 1:2]
                )

            # ---- phase C: query
            pq = pp.tile([33, TCH, TCHUNK], f32, name="pq")
            for j in range(NGT):
                dq = qp.tile([P, TL], f32, name="dq")
                nc.vector.tensor_scalar(
                    dq[:], xt_sb[:], tcol_sb[:, j : j + 1], None, OP.subtract
                )
                dqa = qp.tile([P, TL], f32, name="dqa")
                _abs_ts(dqa, dq)
                bs = qp.tile([P, TL], f32, name="bs")
                nc.scalar.activation(bs[:], dqa[:], AF.Exp, scale=scale_q)
                for c in range(TCH):
                    nc.tensor.matmul(
                        pq[:, c, :],
                        gencT[:, 33 * j : 33 * j + 33],
                        bs[:, TCHUNK * c : TCHUNK * (c + 1)],
                        start=(j == 0),
                        stop=(j == NGT - 1),
                    )

            means_sb = cp.tile([1, TL], f32)
            stds_sb = cp.tile([1, TL], f32)
            means_v = means_sb[0:1, :].rearrange("p (c n) -> p c n", c=TCH)
            stds_v = stds_sb[0:1, :].rearrange("p (c n) -> p c n", c=TCH)
            nc.vector.tensor_copy(means_v, pq[0:1, :, :])
            nc.scalar.activation(stds_v, pq[32:33, :, :], AF.Exp)
            nc.sync.dma_start(out_d[0:1, :], means_sb[:])
            nc.sync.dma_start(out_d[1:2, :], stds_sb[:])

    if not nc.is_finalized():
        nc.finalize()
    return nc


def _prep_inputs(X_c, y_c, X_t, t, log_l, log_query_l, w1, b1, w2, b2, w3, b3):
    xc = np.ascontiguousarray(X_c[:, 0], dtype=np.float32)
    yc = np.ascontiguousarray(y_c[:, 0], dtype=np.float32)
    xt = np.ascontiguousarray(X_t[:, 0], dtype=np.float32)
    tg = np.ascontiguousarray(t[:, 0], dtype=np.float32)
    G = tg.shape[0]
    t0 = float(tg[0])
    dt = GRID_SPACING
    l = float(np.exp(np.asarray(log_l, dtype=np.float64)[0]))
    ql = float(np.exp(np.asarray(log_query_l, dtype=np.float64)[0]))

    T = xt.shape[0]
    assert T % NCORES == 0
    TL = T // NCORES
    assert TL % TCHUNK == 0

    order = np.argsort(xt, kind="stable")
    xt_s = xt[order]

    # per-core window of grid rows: targets +- PAD, +- HALO, small margin
    los, his = [], []
    for c in range(NCORES):
        lo_t = float(xt_s[c * TL])
        hi_t = float(xt_s[(c + 1) * TL - 1])
        los.append(int(np.floor((lo_t - PAD - t0) / dt)) - 2)
        his.append(int(np.ceil((hi_t + PAD - t0) / dt)) + 2)
    W = _round_up(max(his[c] - los[c] + 1 + 2 * HALO for c in range(NCORES)), P)
    assert W <= G, f"window {W} exceeds grid {G}"
    starts = [min(max(los[c] - HALO, 0), G - W) for c in range(NCORES)]

    # per-core context shard: points within the window band
    ncs = []
    sels = []
    for c in range(NCORES):
        t_lo = tg[starts[c]]
        t_hi = tg[starts[c] + W - 1]
        sel = np.where((xc >= t_lo - PAD) & (xc <= t_hi + PAD))[0]
        sels.append(sel)
        ncs.append(len(sel))
    NC_LOC = max(P, _round_up(max(ncs), P))
    NCT = NC_LOC // P
    NGT = W // P

    in_maps = []
    w1T = np.ascontiguousarray(
        w1.astype(np.float32).transpose(1, 2, 0).reshape(2, 5 * 32)
    )
    w2T = np.ascontiguousarray(
        w2.astype(np.float32).transpose(1, 2, 0).reshape(32, 5 * 32)
    )
    w3T = np.ascontiguousarray(
        w3.astype(np.float32).transpose(1, 2, 0).reshape(32, 5 * 2)
    )
    b1c = np.ascontiguousarray(b1.astype(np.float32).reshape(32, 1))
    b2c = np.ascontiguousarray(b2.astype(np.float32).reshape(32, 1))
    b3r = np.ascontiguousarray(
        np.broadcast_to(b3.astype(np.float32).reshape(1, 2), (P, 2))
    )
    for c in range(NCORES):
        sel = sels[c]
        xc_pad = np.full(NC_LOC, SENTINEL, dtype=np.float32)
        yc_pad = np.zeros(NC_LOC, dtype=np.float32)
        ones_pad = np.zeros(NC_LOC, dtype=np.float32)
        xc_pad[: len(sel)] = xc[sel]
        yc_pad[: len(sel)] = yc[sel]
        ones_pad[: len(sel)] = 1.0
        # [128, NCT] column i <- contexts [i*128, (i+1)*128)
        xc_cols = np.ascontiguousarray(xc_pad.reshape(NCT, P).T)
        yc1 = np.zeros((P, 33 * NCT), dtype=np.float32)
        yc1[:, 0::33] = yc_pad.reshape(NCT, P).T
        yc1[:, 32::33] = ones_pad.reshape(NCT, P).T

        t_win = tg[starts[c] : starts[c] + W]
        t_bcast = np.ascontiguousarray(np.broadcast_to(t_win, (P, W)))
        t_cols = np.ascontiguousarray(t_win.reshape(NGT, P).T)

        xt_loc = xt_s[c * TL : (c + 1) * TL]
        xt_bcast = np.ascontiguousarray(np.broadcast_to(xt_loc, (P, TL)))

        in_maps.append(
            dict(
                xc_cols=xc_cols,
                yc1=np.ascontiguousarray(yc1),
                t_bcast=t_bcast,
                t_cols=t_cols,
                xt_bcast=xt_bcast,
                w1T=w1T,
                w2T=w2T,
                w3T=w3T,
                b1c=b1c,
                b2c=b2c,
                b3r=b3r,
            )
        )

    meta = dict(
        W=W, NCT=NCT, NGT=NGT, TL=TL,
        scale_c=-0.5 / l, scale_q=-0.5 / ql,
        order=order, T=T,
    )
    return in_maps, meta


def kernel(X_c, y_c, X_t, t, log_l, log_query_l, w1, b1, w2, b2, w3, b3):
    global LAST_RESULT
    in_maps, meta = _prep_inputs(
        X_c, y_c, X_t, t, log_l, log_query_l, w1, b1, w2, b2, w3, b3
    )
    sim = os.environ.get("CONVCNP_SIM") == "1"
    nc = _build_program(
        meta["W"], meta["NCT"], meta["NGT"], meta["TL"],
        meta["scale_c"], meta["scale_q"], sim_abs=sim,
    )

    if sim:
        from concourse.bass_interp import CoreSim

        outs = []
        for c in range(NCORES):
            sim = CoreSim(nc)
            for name, arr in in_maps[c].items():
                sim.tensor(name)[:] = arr
            sim.simulate()
            outs.append(np.array(sim.tensor("out")))
    else:
        from concourse.bass_utils import run_bass_kernel_spmd

        res = run_bass_kernel_spmd(nc, in_maps, core_ids=list(range(NCORES)))
        LAST_RESULT = res
        outs = [res.results[c]["out"] for c in range(NCORES)]

    out_sorted = np.concatenate([o.T for o in outs], axis=0)  # [T, 2]
    out = np.empty((meta["T"], 2), dtype=np.float32)
    out[meta["order"]] = out_sorted
    return out
